# revision 8
# baseline (speedup 1.0000x reference)
"""DEDICOM decoder forward on 8 Trainium2 NeuronCores.

Math per relation k (k=0..7):
    M_k = diag(d_k) @ G @ diag(d_k)                  (64x64, host-precomputed)
    out[k, n] = sigmoid( (row_n @ M_k) . col_n )

Device algorithm (data-parallel over N across 8 cores; per core SHARD=62500
samples padded to 512*128=65536; sample s lives at (p=s//512, t=s%512)):

  Stage 1 (PE): per tile t (128 samples) and k-pair p in 0..3:
      Y^T[(kappa,j), n] = sum_i Mpair_p[i, (kappa,j)] * rowT[i, n]
    i.e. matmul(lhsT=Mquad[:,p,:] [64,128], rhs=rowT_g[:,t,:] [64,128])
    -> PSUM f32 [128, 128], four pairs packed in one [128,512] bank slice.

  Col-multiply U^T = Y^T * colT2 (colT duplicated on both partition halves),
  split across three engines per 2-tile block:
      ACT   : bridge pairs 0-1  PSUM f32 -> SBUF bf16
      DVE   : bf16 mult pairs 0-1; f32 PSUM-direct mult pair 2
      Pool  : f32 PSUM-direct mult pair 3

  Stage 2 (PE): reduce over j=64 per kappa via constant selection matrix:
      matmul(lhsT=U^T[:,b,pair,:] [128,128], rhs=sel [128,2])
    -> rec PSUM [128, 2] slices of a [128,64,8] bank (k = 2*pair+kappa).

  ACT: sigmoid per 64-tile group [128,64,8] -> SBUF f32; DMA out.
"""

import sys

sys.path.insert(0, "/opt/trn_rl_repo")

import numpy as np
import ml_dtypes

import concourse.bass as bass
import concourse.bacc as bacc
import concourse.mybir as mybir
from concourse import tile
from concourse.bass_utils import run_bass_kernel_spmd

N, D, R = 500000, 64, 8
NCORES = 8
SHARD = N // NCORES            # 62500
TPP = 512                      # samples per partition; 512*128 = 65536 >= 62500
SHARD_PAD = TPP * 128
W = 64                         # tiles per group (rec PSUM bank = [128,64,8] f32)
NGROUPS = TPP // W             # 8
BF16 = mybir.dt.bfloat16
F32 = mybir.dt.float32

_CACHE: dict = {}


def _build_program():
    if "nc" in _CACHE:
        return _CACHE["nc"]

    nc = bacc.Bacc(
        "TRN2", target_bir_lowering=False, debug=False, num_devices=NCORES
    )

    rowT_d = nc.dram_tensor("rowt", [D, TPP, 128], BF16, kind="ExternalInput")
    colT_d = nc.dram_tensor("colt", [D, TPP, 128], BF16, kind="ExternalInput")
    mq_d = nc.dram_tensor("mquad", [128, 2 * 128], BF16, kind="ExternalInput")
    sel_d = nc.dram_tensor("sel", [128, 2], BF16, kind="ExternalInput")
    out_d = nc.dram_tensor("out", [SHARD_PAD, R], F32, kind="ExternalOutput")

    out_v = out_d.ap().rearrange("(p t) k -> p t k", p=128)

    MULT = mybir.AluOpType.mult

    with tile.TileContext(nc) as tc:
        with (
            tc.tile_pool(name="const", bufs=1) as cpool,
            tc.tile_pool(name="io", bufs=2) as iopool,
            tc.tile_pool(name="work", bufs=4) as wpool,
            tc.tile_pool(name="psum_y0", bufs=2, space="PSUM") as py0_pool,
            tc.tile_pool(name="psum_y1", bufs=2, space="PSUM") as py1_pool,
            tc.tile_pool(name="psum_r", bufs=2, space="PSUM") as pr_pool,
        ):
            # mquad2[64h+i, q, (kappa,j)] = M_{2*(2q+h)+kappa}[i, j]
            mquad = cpool.tile([128, 2, 128], BF16, tag="mquad")
            sel = cpool.tile([128, 2], BF16, tag="sel")
            nc.sync.dma_start(
                mquad[:].rearrange("d q j -> d (q j)"), mq_d.ap()
            )
            nc.sync.dma_start(sel[:], sel_d.ap())

            for g in range(NGROUPS):
                t0 = g * W
                rowT_g = iopool.tile([128, W, 128], BF16, tag="rowT_g")
                colT2_g = iopool.tile([128, W, 128], BF16, tag="colT2_g")
                # rowT/colT duplicated onto both partition halves (PE row tiling)
                nc.sync.dma_start(
                    rowT_g[0:64, :, :], rowT_d.ap()[:, t0 : t0 + W, :]
                )
                nc.sync.dma_start(
                    rowT_g[64:128, :, :], rowT_d.ap()[:, t0 : t0 + W, :]
                )
                nc.sync.dma_start(
                    colT2_g[0:64, :, :], colT_d.ap()[:, t0 : t0 + W, :]
                )
                nc.sync.dma_start(
                    colT2_g[64:128, :, :], colT_d.ap()[:, t0 : t0 + W, :]
                )

                rec_ps = pr_pool.tile([128, W, R], F32, tag="rec")

                for b0 in range(0, W, 2):
                    # stage 1, row-tiled: h=0/1 halves run concurrently on
                    # the PE; each half owns its own PSUM bank (y0 / y1).
                    # y_h[:, b, q, :] holds pair p = 2q+h.
                    y0 = py0_pool.tile([128, 2, 2, 128], F32, tag="y0")
                    y1 = py1_pool.tile([128, 2, 2, 128], F32, tag="y1")
                    for q in range(2):
                        for b in range(2):
                            t = b0 + b
                            for h, yh in ((0, y0), (1, y1)):
                                nc.tensor.matmul(
                                    yh[:, b, q, :],
                                    mquad[64 * h : 64 * h + 64, q, :],
                                    rowT_g[64 * h : 64 * h + 64, t, :],
                                )

                    colb = colT2_g[:, b0 : b0 + 2, :]
                    # ut free index m: 0=pair0, 1=pair2, 2=pair1, 3=pair3
                    ut = wpool.tile([128, 2, 4, 128], BF16, tag="ut")

                    # ACT: bridge pairs 1,3 (=y1) to bf16
                    ybf = wpool.tile([128, 2, 2, 128], BF16, tag="ybf")
                    nc.scalar.copy(ybf[:], y1[:])
                    # DVE: f32 PSUM-direct mult pairs 0,2 (=y0)
                    nc.vector.tensor_tensor(
                        out=ut[:, :, 0:2, :],
                        in0=y0[:],
                        in1=colb.unsqueeze(2).broadcast_to([128, 2, 2, 128]),
                        op=MULT,
                    )
                    # DVE: bf16 mult pair 1
                    nc.vector.tensor_tensor(
                        out=ut[:, :, 2, :],
                        in0=ybf[:, :, 0, :],
                        in1=colb,
                        op=MULT,
                    )
                    # Pool: bf16 mult pair 3 (SBUF only)
                    nc.gpsimd.tensor_tensor(
                        out=ut[:, :, 3, :],
                        in0=ybf[:, :, 1, :],
                        in1=colb,
                        op=MULT,
                    )

                    # stage 2: PE reduce over j via selection matrix
                    for b in range(2):
                        t = b0 + b
                        for m, p in ((0, 0), (1, 2), (2, 1), (3, 3)):
                            nc.tensor.matmul(
                                rec_ps[:, t, 2 * p : 2 * p + 2],
                                ut[:, b, m, :],
                                sel[:],
                            )

                sig_g = wpool.tile([128, W, R], F32, tag="sig")
                nc.scalar.activation(
                    sig_g[:],
                    rec_ps[:],
                    mybir.ActivationFunctionType.Sigmoid,
                )
                nc.sync.dma_start(out_v[:, t0 : t0 + W, :], sig_g[:])

    nc.compile()
    _CACHE["nc"] = nc
    return nc


def _prep_inputs(inputs_row, inputs_col, global_interaction, local_variation):
    d = np.asarray(local_variation, np.float32)
    g = np.asarray(global_interaction, np.float32)
    # Mquad[i, p, (kappa, j)] = M_{2p+kappa}[i, j] = d[k,i]*G[i,j]*d[k,j]
    mk = np.einsum("ki,ij,kj->kij", d, g, d)            # [8, 64, 64]
    # mq2[64h+i, (q, kappa, j)] = M_{2*(2q+h)+kappa}[i, j]
    mq2 = np.zeros((128, 2, 2, D), np.float32)
    for h in range(2):
        for q in range(2):
            for kap in range(2):
                mq2[64 * h : 64 * h + 64, q, kap, :] = mk[2 * (2 * q + h) + kap]
    mquad = mq2.reshape(128, 2 * 128).astype(ml_dtypes.bfloat16)
    sel = np.zeros((128, 2), np.float32)
    sel[0:64, 0] = 1.0
    sel[64:128, 1] = 1.0
    sel = sel.astype(ml_dtypes.bfloat16)

    pad = SHARD_PAD - SHARD
    in_maps = []
    for c in range(NCORES):
        sl = slice(c * SHARD, (c + 1) * SHARD)
        rr = np.concatenate(
            [np.asarray(inputs_row[sl], np.float32), np.zeros((pad, D), np.float32)]
        ).astype(ml_dtypes.bfloat16)
        cc = np.concatenate(
            [np.asarray(inputs_col[sl], np.float32), np.zeros((pad, D), np.float32)]
        ).astype(ml_dtypes.bfloat16)
        rowt = np.ascontiguousarray(rr.reshape(128, TPP, D).transpose(2, 1, 0))
        colt = np.ascontiguousarray(cc.reshape(128, TPP, D).transpose(2, 1, 0))
        in_maps.append(
            {"rowt": rowt, "colt": colt, "mquad": mquad, "sel": sel}
        )
    return in_maps


def kernel(inputs_row, inputs_col, global_interaction, local_variation):
    nc = _build_program()
    in_maps = _prep_inputs(
        inputs_row, inputs_col, global_interaction, local_variation
    )
    res = run_bass_kernel_spmd(nc, in_maps, list(range(NCORES)))
    outs = [res.results[c]["out"][:SHARD] for c in range(NCORES)]
    full = np.concatenate(outs, axis=0)  # [N, 8] f32
    return np.ascontiguousarray(full.T)  # [8, N]


if __name__ == "__main__":
    rng = np.random.default_rng(0)
    inputs = {
        "inputs_row": rng.standard_normal((N, D), dtype=np.float32),
        "inputs_col": rng.standard_normal((N, D), dtype=np.float32),
        "global_interaction": rng.uniform(-0.2, 0.2, (D, D)).astype(np.float32),
        "local_variation": rng.uniform(-0.3, 0.3, (R, D)).astype(np.float32),
    }
    out = kernel(**inputs)
    print("out", out.shape, out.dtype, out[:, :3])
